# revision 1
# baseline (speedup 1.0000x reference)
"""GPTQ-Marlin sparse MoE layer for 8 Trainium2 NeuronCores.

Strategy (expert-parallel, host-side dispatch):
  - Router (softmax + top-2 + renormalize) is computed with the same jax ops
    as the reference so expert selection matches bit-for-bit.
  - Tokens are gathered per expert on the host (avg ~T*TOPK/E = 1024 tokens
    per expert), padded to a common multiple-of-128 capacity C so all 8
    cores run one SPMD program: core e owns expert e.
  - GPTQ int4 codes are dequantized to bf16 on the host; each core streams
    its expert's W1 [D,2F] / W2 [F,D] from HBM exactly once.
  - Device kernel per core: h = x @ W1 (transposed layout), act = silu(gate)*up,
    y = act @ W2 — all bf16 matmuls with fp32 PSUM accumulation.
  - Host applies the top-k coefficients during the scatter-add combine.

Everything runs transposed (activations keep tokens on the free dim) so no
on-device transposes are needed:
  mm1: psum[n, t] = sum_k W1[k, n] * xT[k, t]   (lhsT = W1 as stored)
  mm2: psum[d, t] = sum_f W2[f, d] * actT[f, t] (lhsT = W2 as stored)
"""

import numpy as np
import ml_dtypes

E, T, D, F, TOPK, GROUP = 8, 4096, 1024, 4096, 2, 128
P = 128
KO1 = D // P          # 8  k-tiles for mm1
FH = F // P           # 32 act tiles (and k-tiles for mm2)
DO = D // P           # 8  output d-tiles
NCH = (2 * F) // 1024  # 8 column chunks of W1 (0..3 gate, 4..7 up)
TC = 512              # token chunk (one PSUM bank of fp32)

LAST_RESULTS = None   # test harness introspection

_BUILD_CACHE = {}


def _route(gating_output):
    """softmax + top-k + renormalize, replicated exactly like the reference."""
    try:
        import jax
        import jax.numpy as jnp

        scores = jax.nn.softmax(jnp.asarray(gating_output, jnp.float32), axis=-1)
        topk_w, topk_ids = jax.lax.top_k(scores, TOPK)
        topk_w = topk_w / jnp.sum(topk_w, axis=-1, keepdims=True)
        return np.asarray(topk_w, np.float32), np.asarray(topk_ids)
    except Exception:
        g = np.asarray(gating_output, np.float32)
        ex = np.exp(g - g.max(axis=-1, keepdims=True))
        s = (ex / ex.sum(axis=-1, keepdims=True)).astype(np.float32)
        ids = np.argsort(-s, axis=-1, kind="stable")[:, :TOPK]
        w = np.take_along_axis(s, ids, axis=-1)
        w = (w / w.sum(axis=-1, keepdims=True)).astype(np.float32)
        return w, ids


def _dequant_bf16(q, s):
    """q: [K, N] int codes, s: [K//GROUP, N] scales -> bf16 [K, N]."""
    w = (np.asarray(q, np.float32) - 8.0) * np.repeat(
        np.asarray(s, np.float32), GROUP, axis=0
    )
    return w.astype(ml_dtypes.bfloat16)


def _build(C, native_silu=True):
    """Build the per-core FFN program for token capacity C (multiple of 128).

    native_silu=False decomposes silu as gate*sigmoid(gate) (CoreSim has no
    Silu LUT); hardware always runs with the native Silu activation.
    """
    import concourse.mybir as mybir
    import concourse.tile as tile
    from concourse import bacc

    nc = bacc.Bacc("TRN2", name="moe_expert_ffn")
    bf16 = mybir.dt.bfloat16
    f32 = mybir.dt.float32

    xT = nc.dram_tensor("xT", [P, KO1, C], bf16, kind="ExternalInput")
    w1 = nc.dram_tensor("w1", [P, NCH, KO1, 1024], bf16, kind="ExternalInput")
    w2 = nc.dram_tensor("w2", [P, DO, FH, P], bf16, kind="ExternalInput")
    yT = nc.dram_tensor("yT", [P, DO, C], f32, kind="ExternalOutput")

    # token chunks (PSUM bank = 512 fp32)
    tcs = []
    t0 = 0
    while t0 < C:
        w_ = min(TC, C - t0)
        tcs.append((t0, w_))
        t0 += w_

    with tile.TileContext(nc) as tc:
        with (
            tc.tile_pool(name="xpool", bufs=1) as xpool,
            tc.tile_pool(name="w1pool", bufs=4) as w1pool,
            tc.tile_pool(name="w2pool", bufs=3) as w2pool,
            tc.tile_pool(name="actpool", bufs=1) as actpool,
            tc.tile_pool(name="sgpool", bufs=4) as sgpool,
            tc.tile_pool(name="ypool", bufs=4) as ypool,
            tc.tile_pool(name="pspool", bufs=4, space="PSUM") as pspool,
            tc.tile_pool(name="psypool", bufs=2, space="PSUM") as psypool,
        ):
            xsb = xpool.tile([P, KO1, C], bf16)
            nc.sync.dma_start(xsb[:], xT[:])
            act = actpool.tile([P, FH, C], bf16)

            # ---- mm1: h^T = W1^T x, then act = silu(gate) * up ----
            for c in range(NCH // 2):
                wg = w1pool.tile([P, KO1, 1024], bf16, tag="w1c")
                nc.sync.dma_start(wg[:], w1[:, c])
                wu = w1pool.tile([P, KO1, 1024], bf16, tag="w1c")
                nc.sync.dma_start(wu[:], w1[:, c + NCH // 2])
                for j in range(8):
                    i = c * 8 + j  # act tile index 0..31
                    for (t0, tw) in tcs:
                        psg = pspool.tile([P, TC], f32, tag="ps")
                        psu = pspool.tile([P, TC], f32, tag="ps")
                        for k in range(KO1):
                            nc.tensor.matmul(
                                psg[:, :tw],
                                wg[:, k, j * P : (j + 1) * P],
                                xsb[:, k, t0 : t0 + tw],
                                start=(k == 0),
                                stop=(k == KO1 - 1),
                            )
                            nc.tensor.matmul(
                                psu[:, :tw],
                                wu[:, k, j * P : (j + 1) * P],
                                xsb[:, k, t0 : t0 + tw],
                                start=(k == 0),
                                stop=(k == KO1 - 1),
                            )
                        sg = sgpool.tile([P, TC], f32, tag="sg")
                        if native_silu:
                            nc.scalar.activation(
                                sg[:, :tw], psg[:, :tw],
                                mybir.ActivationFunctionType.Silu,
                            )
                        else:
                            nc.scalar.activation(
                                sg[:, :tw], psg[:, :tw],
                                mybir.ActivationFunctionType.Sigmoid,
                            )
                            nc.vector.tensor_tensor(
                                sg[:, :tw],
                                sg[:, :tw],
                                psg[:, :tw],
                                mybir.AluOpType.mult,
                            )
                        nc.vector.tensor_tensor(
                            act[:, i, t0 : t0 + tw],
                            sg[:, :tw],
                            psu[:, :tw],
                            mybir.AluOpType.mult,
                        )

            # ---- mm2: y^T = W2^T act ----
            for d in range(DO):
                ws = w2pool.tile([P, FH, P], bf16, tag="w2s")
                nc.sync.dma_start(ws[:], w2[:, d])
                for (t0, tw) in tcs:
                    psy = psypool.tile([P, TC], f32, tag="psy")
                    for k2 in range(FH):
                        nc.tensor.matmul(
                            psy[:, :tw],
                            ws[:, k2],
                            act[:, k2, t0 : t0 + tw],
                            start=(k2 == 0),
                            stop=(k2 == FH - 1),
                        )
                    yo = ypool.tile([P, TC], f32, tag="yo")
                    nc.scalar.copy(yo[:, :tw], psy[:, :tw])
                    nc.sync.dma_start(yT[:, d, t0 : t0 + tw], yo[:, :tw])
    return nc


def kernel(x, gating_output, w1_q, w2_q, w1_scale, w2_scale):
    global LAST_RESULTS
    from concourse.bass_utils import run_bass_kernel_spmd

    x = np.asarray(x, np.float32)
    w1_q = np.asarray(w1_q)
    w2_q = np.asarray(w2_q)
    w1_scale = np.asarray(w1_scale, np.float32)
    w2_scale = np.asarray(w2_scale, np.float32)

    topk_w, topk_ids = _route(gating_output)

    token_lists, coefs = [], []
    for e in range(E):
        mask = topk_ids == e
        tok = np.nonzero(mask.any(axis=1))[0]
        cf = np.where(mask, topk_w, 0.0).sum(axis=1)[tok].astype(np.float32)
        token_lists.append(tok)
        coefs.append(cf)

    cmax = max(len(t) for t in token_lists)
    C = max(P, ((cmax + P - 1) // P) * P)

    key = C
    if key not in _BUILD_CACHE:
        nc = _build(C)
        nc.finalize()
        _BUILD_CACHE[key] = nc
    nc = _BUILD_CACHE[key]

    in_maps = []
    for e in range(E):
        tok = token_lists[e]
        xe = np.zeros((C, D), np.float32)
        xe[: len(tok)] = x[tok]
        # [C, D] -> [P, KO1, C] with xT[p, k, t] = x[t, k*P + p]
        xTh = np.ascontiguousarray(
            xe.T.reshape(KO1, P, C).transpose(1, 0, 2)
        ).astype(ml_dtypes.bfloat16)

        w1d = _dequant_bf16(w1_q[e], w1_scale[e])   # [D, 2F]
        w1h = np.ascontiguousarray(
            w1d.reshape(KO1, P, NCH, 1024).transpose(1, 2, 0, 3)
        )
        w2d = _dequant_bf16(w2_q[e], w2_scale[e])   # [F, D]
        w2h = np.ascontiguousarray(
            w2d.reshape(FH, P, DO, P).transpose(1, 2, 0, 3)
        )
        in_maps.append({"xT": xTh, "w1": w1h, "w2": w2h})

    LAST_RESULTS = run_bass_kernel_spmd(nc, in_maps, core_ids=list(range(E)))

    out = np.zeros((T, D), np.float32)
    for e in range(E):
        yTe = LAST_RESULTS.results[e]["yT"]          # [P, DO, C] f32
        y = yTe.transpose(1, 0, 2).reshape(D, C).T   # [C, D]
        tok = token_lists[e]
        out[tok] += coefs[e][:, None] * y[: len(tok)]
    return out



# revision 5
# speedup vs baseline: 1.0543x; 1.0543x over previous
"""GPTQ-Marlin sparse MoE layer for 8 Trainium2 NeuronCores.

Strategy (expert-parallel + d_ff-slice rebalancing, host-side dispatch):
  - Router (softmax + top-2 + renormalize) replicates the reference with the
    same jax ops so expert selection matches bit-for-bit.
  - Phase 1: core e owns expert e and runs the FFN for the first C1 tokens of
    that expert (C1 chosen to minimize per-core work; NOT padded to 128).
  - Phase 2: the leftover tokens (experts with more than C1 tokens) are cut
    into 128-token blocks x 8 d_ff slices of 512 -> identical "units"
    distributed round-robin so every core gets the same number. A unit
    computes gate/up/act for its f-slice and a partial down-projection; the
    host sums the slice partials during the scatter-add combine (free).
  - GPTQ int4 codes are dequantized to bf16 on the host; weights stream from
    HBM in fine-grained tiles so the first matmul starts ~5us into the run.
  - All matmuls run transposed (tokens on the free dim): bf16 with fp32 PSUM.

Per-core tensor time ~ (C1 + 16 * units_per_core) token-equivalents vs the
naive max-expert padding; for typical routing this is ~1058 vs 1152.
"""

import numpy as np
import ml_dtypes

E, T, D, F, TOPK, GROUP = 8, 4096, 1024, 8192 // 2, 2, 128
P = 128
KO1 = D // P            # 8 k-tiles for mm1
FH = F // P             # 32 act tiles (k-tiles for mm2)
DO = D // P             # 8 output d-tiles
NJ = (2 * F) // P // 2  # 32 gate/up column-tile pairs for mm1
TC = 512                # token chunk (one PSUM bank of fp32)
FSL = 512               # phase-2 d_ff slice width
NSL = F // FSL          # 8 slices per expert
FH2 = FSL // P          # 4 act tiles per slice
NJ2 = FSL // P          # 4 gate/up pairs per slice

LAST_RESULTS = None     # test harness introspection

_BUILD_CACHE = {}


def _route(gating_output):
    """softmax + top-k + renormalize, replicated exactly like the reference."""
    try:
        import jax
        import jax.numpy as jnp

        scores = jax.nn.softmax(jnp.asarray(gating_output, jnp.float32), axis=-1)
        topk_w, topk_ids = jax.lax.top_k(scores, TOPK)
        topk_w = topk_w / jnp.sum(topk_w, axis=-1, keepdims=True)
        return np.asarray(topk_w, np.float32), np.asarray(topk_ids)
    except Exception:
        g = np.asarray(gating_output, np.float32)
        ex = np.exp(g - g.max(axis=-1, keepdims=True))
        s = (ex / ex.sum(axis=-1, keepdims=True)).astype(np.float32)
        ids = np.argsort(-s, axis=-1, kind="stable")[:, :TOPK]
        w = np.take_along_axis(s, ids, axis=-1)
        w = (w / w.sum(axis=-1, keepdims=True)).astype(np.float32)
        return w, ids


def _dequant_bf16(q, s):
    """q: [K, N] int codes, s: [K//GROUP, N] scales -> bf16 [K, N]."""
    w = (np.asarray(q, np.float32) - 8.0) * np.repeat(
        np.asarray(s, np.float32), GROUP, axis=0
    )
    return w.astype(ml_dtypes.bfloat16)


def _chunks(C):
    out, t0 = [], 0
    while t0 < C:
        w = min(TC, C - t0)
        out.append((t0, w))
        t0 += w
    return out


def _build(C1, UPC):
    """Per-core program: phase-1 FFN for C1 tokens of one expert, then UPC
    phase-2 units (128 tokens x 512 d_ff slice each, possibly zero-padded)."""
    import concourse.mybir as mybir
    import concourse.tile as tile
    from concourse import bacc

    nc = bacc.Bacc("TRN2", name="moe_expert_ffn")
    bf16 = mybir.dt.bfloat16
    f32 = mybir.dt.float32

    tcs = _chunks(C1)
    NCHK = len(tcs)

    xT = nc.dram_tensor("xT", [P, KO1, C1], bf16, kind="ExternalInput")
    # [p, jj, gate/up, k, col]
    w1 = nc.dram_tensor("w1", [P, NJ, 2, KO1, P], bf16, kind="ExternalInput")
    w2 = nc.dram_tensor("w2", [P, DO, FH, P], bf16, kind="ExternalInput")
    yT = nc.dram_tensor("yT", [P, DO, C1], f32, kind="ExternalOutput")
    if UPC:
        x2T = nc.dram_tensor("x2T", [P, KO1, UPC * P], bf16, kind="ExternalInput")
        # [p, unit, jj2, gate/up, k, col]
        w1s = nc.dram_tensor("w1s", [P, UPC, NJ2, 2, KO1, P], bf16,
                             kind="ExternalInput")
        # [p, unit, k2, d, col]
        w2s = nc.dram_tensor("w2s", [P, UPC, FH2, DO, P], bf16,
                             kind="ExternalInput")
        y2T = nc.dram_tensor("y2T", [P, UPC, DO, P], f32, kind="ExternalOutput")

    with tile.TileContext(nc) as tc:
        with (
            tc.tile_pool(name="xpool", bufs=1) as xpool,
            tc.tile_pool(name="w1pool", bufs=4) as w1pool,
            tc.tile_pool(name="w2pool", bufs=2) as w2pool,
            tc.tile_pool(name="actpool", bufs=1) as actpool,
            tc.tile_pool(name="sgpool", bufs=4) as sgpool,
            tc.tile_pool(name="ypool", bufs=4) as ypool,
            tc.tile_pool(name="pspool", bufs=5, space="PSUM") as pspool,
            tc.tile_pool(name="psypool", bufs=2, space="PSUM") as psypool,
        ):
            # x streams in per token-chunk so the first matmul only waits for
            # chunk 0 plus the first weight tile.
            xcs = []
            for ci, (t0, tw) in enumerate(tcs):
                xc = xpool.tile([P, KO1, tw], bf16, tag=f"x{ci}")
                nc.sync.dma_start(xc[:], xT[:, :, t0:t0 + tw])
                xcs.append(xc)

            act = actpool.tile([P, FH, C1], bf16)

            # ---- phase 1 mm1: h^T = W1^T x; act = silu(gate) * up ----
            for jj in range(NJ):
                wg = w1pool.tile([P, KO1, P], bf16, tag="w1t")
                nc.sync.dma_start(wg[:], w1[:, jj, 0])
                wu = w1pool.tile([P, KO1, P], bf16, tag="w1t")
                nc.sync.dma_start(wu[:], w1[:, jj, 1])
                for ci, (t0, tw) in enumerate(tcs):
                    psg = pspool.tile([P, TC], f32, tag="ps")
                    psu = pspool.tile([P, TC], f32, tag="ps")
                    for k in range(KO1):
                        nc.tensor.matmul(
                            psg[:, :tw], wg[:, k], xcs[ci][:, k],
                            start=(k == 0), stop=(k == KO1 - 1),
                        )
                    for k in range(KO1):
                        nc.tensor.matmul(
                            psu[:, :tw], wu[:, k], xcs[ci][:, k],
                            start=(k == 0), stop=(k == KO1 - 1),
                        )
                    sg = sgpool.tile([P, TC], f32, tag="sg")
                    nc.scalar.activation(
                        sg[:, :tw], psg[:, :tw],
                        mybir.ActivationFunctionType.Silu,
                    )
                    nc.vector.tensor_tensor(
                        act[:, jj, t0:t0 + tw], sg[:, :tw], psu[:, :tw],
                        mybir.AluOpType.mult,
                    )

            # ---- phase 1 mm2: y^T = W2^T act ----
            for d in range(DO):
                ws = w2pool.tile([P, FH, P], bf16, tag="w2t")
                nc.sync.dma_start(ws[:], w2[:, d])
                for ci, (t0, tw) in enumerate(tcs):
                    psy = psypool.tile([P, TC], f32, tag="psy")
                    for k2 in range(FH):
                        nc.tensor.matmul(
                            psy[:, :tw], ws[:, k2], act[:, k2, t0:t0 + tw],
                            start=(k2 == 0), stop=(k2 == FH - 1),
                        )
                    yo = ypool.tile([P, TC], f32, tag="yo")
                    nc.scalar.copy(yo[:, :tw], psy[:, :tw])
                    nc.sync.dma_start(yT[:, d, t0:t0 + tw], yo[:, :tw])

            # ---- phase 2: leftover-token units (128 tok x 512 d_ff) ----
            if UPC:
                x2 = xpool.tile([P, KO1, UPC * P], bf16, tag="x2")
                nc.sync.dma_start(x2[:], x2T[:])
                for u in range(UPC):
                    w1u = w1pool.tile([P, NJ2, 2, KO1, P], bf16, tag="w1s", bufs=2)
                    nc.sync.dma_start(w1u[:], w1s[:, u])
                    w2u = w2pool.tile([P, FH2, DO, P], bf16, tag="w2s")
                    nc.sync.dma_start(w2u[:], w2s[:, u])
                    act2 = actpool.tile([P, FH2, P], bf16, tag="act2", bufs=2)
                    xu = x2[:, :, u * P:(u + 1) * P]
                    for jj in range(NJ2):
                        psg = pspool.tile([P, TC], f32, tag="ps")
                        psu = pspool.tile([P, TC], f32, tag="ps")
                        for k in range(KO1):
                            nc.tensor.matmul(
                                psg[:, :P], w1u[:, jj, 0, k], xu[:, k],
                                start=(k == 0), stop=(k == KO1 - 1),
                            )
                        for k in range(KO1):
                            nc.tensor.matmul(
                                psu[:, :P], w1u[:, jj, 1, k], xu[:, k],
                                start=(k == 0), stop=(k == KO1 - 1),
                            )
                        sg = sgpool.tile([P, TC], f32, tag="sg")
                        nc.scalar.activation(
                            sg[:, :P], psg[:, :P],
                            mybir.ActivationFunctionType.Silu,
                        )
                        nc.vector.tensor_tensor(
                            act2[:, jj], sg[:, :P], psu[:, :P],
                            mybir.AluOpType.mult,
                        )
                    for d in range(DO):
                        psy = psypool.tile([P, TC], f32, tag="psy")
                        for k2 in range(FH2):
                            nc.tensor.matmul(
                                psy[:, :P], w2u[:, k2, d], act2[:, k2],
                                start=(k2 == 0), stop=(k2 == FH2 - 1),
                            )
                        yo = ypool.tile([P, TC], f32, tag="yo")
                        nc.scalar.copy(yo[:, :P], psy[:, :P])
                        nc.sync.dma_start(y2T[:, u, d], yo[:, :P])
    return nc


def _plan(counts):
    """Pick C1 and the phase-2 unit list minimizing per-core token-equivs."""
    cmax = max(counts)
    best = None
    for C1 in range(min(P, cmax), cmax + 1):
        blocks = sum((max(c - C1, 0) + P - 1) // P for c in counts)
        units = blocks * NSL
        upc = (units + E - 1) // E
        cost = C1 + (P * FSL // F) * upc  # C1 + 16 * units-per-core
        if best is None or cost < best[0] or (cost == best[0] and C1 > best[1]):
            best = (cost, C1, upc)
    _, C1, upc = best
    return C1, upc


def _pack_w1_phase1(w1d):
    # w1d [D, 2F] -> [P, NJ, 2, KO1, P]
    g = w1d[:, :F].reshape(KO1, P, NJ, P)       # [k, p, jj, c]
    u = w1d[:, F:].reshape(KO1, P, NJ, P)
    out = np.stack([g.transpose(1, 2, 0, 3), u.transpose(1, 2, 0, 3)], axis=2)
    return np.ascontiguousarray(out)            # [p, jj, gu, k, c]


def _pack_w2_phase1(w2d):
    # w2d [F, D] -> [P, DO, FH, P]
    return np.ascontiguousarray(w2d.reshape(FH, P, DO, P).transpose(1, 2, 0, 3))


def _pack_xT(xe, C):
    # xe [C, D] -> [P, KO1, C]
    return np.ascontiguousarray(xe.T.reshape(KO1, P, C).transpose(1, 0, 2))


def kernel(x, gating_output, w1_q, w2_q, w1_scale, w2_scale):
    global LAST_RESULTS
    from concourse.bass_utils import run_bass_kernel_spmd

    x = np.asarray(x, np.float32)
    w1_q = np.asarray(w1_q)
    w2_q = np.asarray(w2_q)
    w1_scale = np.asarray(w1_scale, np.float32)
    w2_scale = np.asarray(w2_scale, np.float32)

    topk_w, topk_ids = _route(gating_output)

    token_lists, coefs = [], []
    for e in range(E):
        mask = topk_ids == e
        tok = np.nonzero(mask.any(axis=1))[0]
        cf = np.where(mask, topk_w, 0.0).sum(axis=1)[tok].astype(np.float32)
        token_lists.append(tok)
        coefs.append(cf)

    counts = [len(t) for t in token_lists]
    C1, UPC = _plan(counts)

    key = (C1, UPC)
    if key not in _BUILD_CACHE:
        nc = _build(C1, UPC)
        nc.finalize()
        _BUILD_CACHE[key] = nc
    nc = _BUILD_CACHE[key]

    # host-side dequant (once per expert)
    w1ds = [_dequant_bf16(w1_q[e], w1_scale[e]) for e in range(E)]  # [D, 2F]
    w2ds = [_dequant_bf16(w2_q[e], w2_scale[e]) for e in range(E)]  # [F, D]

    # phase-2 unit list: (expert, block_start_in_tok_list, slice)
    units = []
    for e in range(E):
        left = counts[e] - C1
        b0 = C1
        while left > 0:
            for s in range(NSL):
                units.append((e, b0, s))
            b0 += P
            left -= P
    per_core_units = [[] for _ in range(E)]
    for i, unit in enumerate(units):
        per_core_units[i % E].append(unit)

    in_maps = []
    for c in range(E):
        tok = token_lists[c][:C1]
        xe = np.zeros((C1, D), np.float32)
        xe[: len(tok)] = x[tok]
        im = {
            "xT": _pack_xT(xe.astype(ml_dtypes.bfloat16), C1),
            "w1": _pack_w1_phase1(w1ds[c]),
            "w2": _pack_w2_phase1(w2ds[c]),
        }
        if UPC:
            x2 = np.zeros((UPC * P, D), np.float32)
            w1s = np.zeros((P, UPC, NJ2, 2, KO1, P), ml_dtypes.bfloat16)
            w2s = np.zeros((P, UPC, FH2, DO, P), ml_dtypes.bfloat16)
            for j, (e, b0, s) in enumerate(per_core_units[c]):
                btok = token_lists[e][b0:b0 + P]
                x2[j * P: j * P + len(btok)] = x[btok]
                w1d = w1ds[e]
                gs = w1d[:, s * FSL:(s + 1) * FSL].reshape(KO1, P, NJ2, P)
                us = w1d[:, F + s * FSL: F + (s + 1) * FSL].reshape(KO1, P, NJ2, P)
                w1s[:, j] = np.stack(
                    [gs.transpose(1, 2, 0, 3), us.transpose(1, 2, 0, 3)], axis=2
                )
                w2sl = w2ds[e][s * FSL:(s + 1) * FSL]  # [FSL, D]
                w2s[:, j] = w2sl.reshape(FH2, P, DO, P).transpose(1, 0, 2, 3)
            im["x2T"] = _pack_xT(x2.astype(ml_dtypes.bfloat16), UPC * P)
            im["w1s"] = w1s
            im["w2s"] = w2s
        in_maps.append(im)

    LAST_RESULTS = run_bass_kernel_spmd(nc, in_maps, core_ids=list(range(E)))

    out = np.zeros((T, D), np.float32)
    for c in range(E):
        yTe = LAST_RESULTS.results[c]["yT"]          # [P, DO, C1] f32
        y = yTe.transpose(1, 0, 2).reshape(D, C1).T  # [C1, D]
        tok = token_lists[c][:C1]
        out[tok] += coefs[c][: len(tok), None] * y[: len(tok)]
        if UPC:
            y2Te = LAST_RESULTS.results[c]["y2T"]    # [P, UPC, DO, P] f32
            for j, (e, b0, s) in enumerate(per_core_units[c]):
                btok = token_lists[e][b0:b0 + P]
                y2 = y2Te[:, j].transpose(1, 0, 2).reshape(D, P).T  # [P, D]
                out[btok] += coefs[e][b0:b0 + len(btok), None] * y2[: len(btok)]
    return out
